# revision 1
# baseline (speedup 1.0000x reference)
"""TRN2 Bass/Tile kernel for NeoTCNAttention (talking-heads attention with
ALiBi + persistent memory), SPMD over 8 NeuronCores.

Sharding: data-parallel over batch N=4 x 2 halves of the query axis
(each core: one batch element, 1024 query positions, full keys/values).
No collectives: every core computes a disjoint slab of the output.

Per-core pipeline (all matmuls are standard full-array ops):
  - activations arrive transposed/fp16 (x^T pair tiles [128=(2 heads x 64
    dims), seq]); Q/K/V projections use host-built block-diag(W^T, W^T)
    weights, one matmul per 512-wide chunk.
  - W_pre (pre-softmax talking heads) is folded into the score matmuls:
    q~[(h,d), (g, qi)] = W_pre[g, h] * q'^T, so for each output bank
    (4 g-heads x 32 q rows) the scores+mix are 4 accumulated pair matmuls
    plus one ALiBi matmul against a distance table (ALiBi slopes folded
    through W_pre into per-g scalars c_g).
  - exp runs on the scalar engine straight out of PSUM with accumulated
    row sums; softmax normalization is folded into the post-softmax
    talking-heads matrix (per-row 1/sum scaling).
  - post-softmax talking heads run as transposing matmuls (exp tile as
    the stationary operand) producing A'^T [k, (g, q)] directly, which
    feeds A@V with v' columns as stationary weights.
  - fc_out accumulates 8 K=64 matmuls with the bias folded in as a
    rank-1 (ones x fc_b) matmul.
"""

import math

import numpy as np

# ---- problem constants (hardcoded per spec) ----
N_BATCH = 4
SEQ = 2048
EMBED = 512
HEADS = 8
HD = 64
NPERS = 16
KT = SEQ + NPERS  # 2064
ALIBI_ALPHA = 1.25
START_I = 1
N_CORES = 8
SQ = 1024  # query positions per core
SCALE = 1.0 / math.sqrt(EMBED)

QB = 32            # query positions per block (bank rows = 4 g x 32 qi)
NQB = SQ // QB     # 32 blocks
QG = 256           # query positions per A@V group
NQG = SQ // QG     # 4 groups
QBPG = QG // QB    # 8 blocks per group
KC = 512           # k chunk width for scores/exp
NKC = SEQ // KC    # 4 (persistent 16 handled separately)
NKJ = SEQ // 128   # 16 mix2/A@V k-subchunks (+1 partial of NPERS)
DIST_W = SEQ + KT  # 4112 distance-table width

_CACHED = {}
_last_in_maps = None


def _host_consts(Wv, Wk, Wq, W_pre, W_post, p_keys, p_values, fc_w, fc_b):
    """Derived constant tensors shipped to the device (layout prep only)."""
    f16 = np.float16
    Z = np.zeros((HD, HD), np.float32)
    c = {}
    c["wq2"] = np.block([[Wq.T, Z], [Z, Wq.T]]).astype(f16)  # [128, 128]
    c["wk2"] = np.block([[Wk.T, Z], [Z, Wk.T]]).astype(f16)
    c["wv2"] = np.block([[Wv.T, Z], [Z, Wv.T]]).astype(f16)
    c["pkT"] = np.ascontiguousarray(p_keys[:, 0, :].T).astype(f16)   # [64, 16]
    c["pv"] = np.ascontiguousarray(p_values[:, 0, :]).astype(f16)    # [16, 64]

    # W_pre scaling columns for the q~ build: col 4*g + p scales pair p's
    # rows (head 2p in partitions 0-63, head 2p+1 in 64-127) for out-head g.
    wpcol = np.zeros((128, HEADS * 4), np.float32)
    for g in range(HEADS):
        for p in range(4):
            wpcol[0:HD, 4 * g + p] = W_pre[g, 2 * p]
            wpcol[HD:128, 4 * g + p] = W_pre[g, 2 * p + 1]
    c["wpcol"] = wpcol

    slopes = 2.0 ** (
        -ALIBI_ALPHA * (np.arange(1, HEADS + 1, dtype=np.float64) + START_I)
    )
    cg_vec = -(W_pre.astype(np.float64) @ slopes)  # per-out-head slope mix

    cgcol = np.zeros((128, 2), np.float32)
    for oi in range(2):
        for ga in range(4):
            cgcol[32 * ga : 32 * (ga + 1), oi] = cg_vec[4 * oi + ga]
    c["cgcol"] = cgcol

    idx = np.arange(QB)
    for ob, gofs in (("A", 0), ("B", 4)):
        w2p = np.zeros((128, 256), np.float32)
        for ga in range(4):
            for g2 in range(HEADS):
                w2p[32 * ga + idx, 32 * g2 + idx] = W_post[g2, gofs + ga]
        c[f"w2p{ob}"] = w2p.astype(np.float32)

    c["fcwT"] = np.ascontiguousarray(fc_w.T).astype(f16)  # [512, 512] rows=(g,d)
    c["fcb"] = fc_b.reshape(1, EMBED).astype(f16)
    return c


def _dist_table(qbase: int) -> np.ndarray:
    """T[32*ga + qi, u] = |qbase + qi - (u - SEQ)| as fp16 (rows replicated
    over the 4 out-heads of a bank; exact for ints <= 2048)."""
    u = np.arange(DIST_W)
    qi = np.arange(QB)
    t = np.abs(qbase + qi[:, None] - (u[None, :] - SEQ)).astype(np.float16)
    return np.tile(t, (4, 1))


def build_bass():
    import concourse.mybir as mybir
    import concourse.tile as tile
    from concourse import bacc
    from contextlib import ExitStack

    f32 = mybir.dt.float32
    f16 = mybir.dt.float16
    bf16 = mybir.dt.bfloat16
    EXP = mybir.ActivationFunctionType.Exp
    X = mybir.AxisListType.X

    nc = bacc.Bacc(
        "TRN2", target_bir_lowering=False, debug=False, num_devices=N_CORES
    )

    # ---- DRAM I/O ----
    qT_d = nc.dram_tensor("qT16", [EMBED, SQ], f16, kind="ExternalInput").ap()
    kT_d = nc.dram_tensor("kT16", [EMBED, SEQ], f16, kind="ExternalInput").ap()
    vT_d = nc.dram_tensor("vT16", [EMBED, SEQ], f16, kind="ExternalInput").ap()
    wq2_d = nc.dram_tensor("wq2", [128, 128], f16, kind="ExternalInput").ap()
    wk2_d = nc.dram_tensor("wk2", [128, 128], f16, kind="ExternalInput").ap()
    wv2_d = nc.dram_tensor("wv2", [128, 128], f16, kind="ExternalInput").ap()
    pkT_d = nc.dram_tensor("pkT", [HD, NPERS], f16, kind="ExternalInput").ap()
    pv_d = nc.dram_tensor("pv", [NPERS, HD], f16, kind="ExternalInput").ap()
    wpcol_d = nc.dram_tensor(
        "wpcol", [128, HEADS * 4], f32, kind="ExternalInput"
    ).ap()
    cgcol_d = nc.dram_tensor("cgcol", [128, 2], f32, kind="ExternalInput").ap()
    w2p_d = {
        nm: nc.dram_tensor(nm, [128, 256], f32, kind="ExternalInput").ap()
        for nm in ("w2pA", "w2pB")
    }
    dist_d = nc.dram_tensor("dist", [128, DIST_W], f16, kind="ExternalInput").ap()
    fcwT_d = nc.dram_tensor("fcwT", [EMBED, EMBED], f16, kind="ExternalInput").ap()
    fcb_d = nc.dram_tensor("fcb", [1, EMBED], f16, kind="ExternalInput").ap()
    out_d = nc.dram_tensor("out", [SQ, EMBED], f32, kind="ExternalOutput").ap()

    def ecopy(eng, out, in_):
        if eng is nc.scalar:
            eng.copy(out, in_)
        else:
            eng.tensor_copy(out, in_)

    with tile.TileContext(nc) as tc, ExitStack() as ctx:
        const_pool = ctx.enter_context(tc.tile_pool(name="const", bufs=1))
        xT_pool = ctx.enter_context(tc.tile_pool(name="xT", bufs=1))
        vs_pool = ctx.enter_context(tc.tile_pool(name="vs", bufs=1))
        qt_pool = ctx.enter_context(tc.tile_pool(name="qt", bufs=1))
        xraw_pool = ctx.enter_context(tc.tile_pool(name="xraw", bufs=3))
        exp_pool = ctx.enter_context(tc.tile_pool(name="expp", bufs=3))
        smx_pool = ctx.enter_context(tc.tile_pool(name="smx", bufs=2))
        aT_pool = ctx.enter_context(tc.tile_pool(name="aTp", bufs=1))
        at_pool = ctx.enter_context(tc.tile_pool(name="atp", bufs=2))
        os_pool = ctx.enter_context(tc.tile_pool(name="osp", bufs=2))

        # 4 PSUM pools x 2 bufs x 1 bank = 8 banks
        pr_ps = ctx.enter_context(tc.tile_pool(name="pr_ps", bufs=1, space="PSUM"))
        en_ps = ctx.enter_context(tc.tile_pool(name="en_ps", bufs=3, space="PSUM"))
        m2_ps = ctx.enter_context(tc.tile_pool(name="m2_ps", bufs=2, space="PSUM"))
        aux_ps = ctx.enter_context(tc.tile_pool(name="aux_ps", bufs=2, space="PSUM"))

        # ---- constants ----
        def cload(ap_d, shape, dtype, nm):
            t = const_pool.tile(shape, dtype, tag=nm, name=nm)
            nc.sync.dma_start(t[:], ap_d)
            return t

        wq2t = cload(wq2_d, [128, 128], f16, "wq2t")
        wk2t = cload(wk2_d, [128, 128], f16, "wk2t")
        wv2t = cload(wv2_d, [128, 128], f16, "wv2t")
        pkTt = cload(pkT_d, [HD, NPERS], f16, "pkTt")
        pvt = cload(pv_d, [NPERS, HD], f16, "pvt")
        wpcolt = cload(wpcol_d, [128, HEADS * 4], f32, "wpcolt")
        cgcolt = cload(cgcol_d, [128, 2], f32, "cgcolt")
        w2pt = {nm: cload(d, [128, 256], f32, nm) for nm, d in w2p_d.items()}
        distt = cload(dist_d, [128, DIST_W], f16, "distt")
        fcw = []
        for cc in range(HEADS):
            t = const_pool.tile([HD, EMBED], f16, tag=f"fcw{cc}", name=f"fcw{cc}")
            nc.sync.dma_start(t[:], fcwT_d[HD * cc : HD * (cc + 1), :])
            fcw.append(t)
        fcbt = cload(fcb_d, [1, EMBED], f16, "fcbt")
        ones1 = const_pool.tile([1, 128], f16, tag="ones1", name="ones1")
        nc.vector.memset(ones1[:], 1.0)

        # ---- persistent activation tiles ----
        qPT = [
            xT_pool.tile([128, SQ], f16, tag=f"qPT{p}", name=f"qPT{p}")
            for p in range(4)
        ]
        kPT = [
            xT_pool.tile([128, KT], f16, tag=f"kPT{p}", name=f"kPT{p}")
            for p in range(4)
        ]
        vS = [
            vs_pool.tile([128, EMBED], f16, tag=f"vS{j}", name=f"vS{j}")
            for j in range(NKJ)
        ]
        vSp = vs_pool.tile([NPERS, EMBED], f16, tag="vSp", name="vSp")
        # q~: per pair, 8 out-head-scaled copies of the group's q'T columns
        qtil = [
            qt_pool.tile([128, HEADS * QG], f16, tag=f"qt{p}", name=f"qt{p}")
            for p in range(4)
        ]

        for p in range(4):
            nc.vector.tensor_copy(kPT[p][0:HD, SEQ:KT], pkTt[:])
            nc.vector.tensor_copy(kPT[p][HD:128, SEQ:KT], pkTt[:])
        for h in range(HEADS):
            nc.vector.tensor_copy(vSp[:, HD * h : HD * (h + 1)], pvt[:])

        # ---- projections (block-diag weights, K=128) ----
        for src_d, nrows, w2, dstT in ((qT_d, SQ, wq2t, qPT), (kT_d, SEQ, wk2t, kPT)):
            for p in range(4):
                rT = xraw_pool.tile([128, SEQ], f16, tag="xr", name="rT")
                nc.sync.dma_start(rT[:, 0:nrows], src_d[128 * p : 128 * (p + 1), :])
                for c in range(nrows // 512):
                    ps = pr_ps.tile([128, 512], f32, tag="pr", name="ps")
                    nc.tensor.matmul(
                        ps[:],
                        lhsT=w2[:],
                        rhs=rT[:, 512 * c : 512 * (c + 1)],
                        start=True,
                        stop=True,
                    )
                    nc.scalar.copy(dstT[p][:, 512 * c : 512 * (c + 1)], ps[:])

        # v: project and transpose back to natural [seq, (h,d)] in one matmul
        # (vT chunk as the stationary operand against block-diag weights).
        for p in range(4):
            rT = xraw_pool.tile([128, SEQ], f16, tag="xr", name="rTv")
            nc.sync.dma_start(rT[:], vT_d[128 * p : 128 * (p + 1), :])
            for j in range(NKJ):
                ps = pr_ps.tile([128, 512], f32, tag="pr", name="psv")
                nc.tensor.matmul(
                    ps[:, 0:128],
                    lhsT=rT[:, 128 * j : 128 * (j + 1)],
                    rhs=wv2t[:],
                    start=True,
                    stop=True,
                )
                nc.scalar.copy(vS[j][:, 128 * p : 128 * (p + 1)], ps[:, 0:128])

        # ---- main attention loop ----
        for qg in range(NQG):
            q0g = qg * QG
            # q~ build: W_pre-scaled q'^T copies; columns ordered
            # (qblock, bank, ga, qi) so each matmul's weights are contiguous
            for p in range(4):
                qv = qtil[p].rearrange(
                    "p (b o a q) -> p b o a q", b=QBPG, o=2, a=4, q=QB
                )
                pv_in = qPT[p][:, q0g : q0g + QG].rearrange(
                    "p (b q) -> p b q", b=QBPG, q=QB
                )
                for g in range(HEADS):
                    nc.vector.tensor_scalar_mul(
                        qv[:, :, g // 4, g % 4, :],
                        pv_in[:],
                        wpcolt[:, 4 * g + p : 4 * g + p + 1],
                    )

            aT = aT_pool.tile([128, NKJ * HEADS * QG], f16, tag="aT", name="aT")
            aTv = aT.rearrange("p (j g q) -> p j g q", j=NKJ, g=HEADS, q=QG)
            aTp = aT_pool.tile([NPERS, HEADS * QG], f16, tag="aTp", name="aTp")
            aTpv = aTp.rearrange("p (g q) -> p g q", g=HEADS, q=QG)

            for qb_i in range(QBPG):
                b = qg * QBPG + qb_i
                qc0 = b * QB  # core-local query offset of this block

                # fused scores + W_pre mix + alibi, then exp
                # bank A: out-heads 0-3 (rows 32*ga + qi), bank B: 4-7
                ex_s = {}
                acc = smx_pool.tile([128, 16], f32, tag="acc", name="acc")
                for ob in ("A", "B"):
                    ex_s[ob] = exp_pool.tile(
                        [128, KT], bf16, tag=f"exp{ob}", name=f"ex_s{ob}"
                    )
                for c in range(NKC + 1):
                    k0 = c * KC
                    w = KC if c < NKC else NPERS
                    has_alibi = c < NKC
                    for oi, ob in enumerate(("A", "B")):
                        gofs = 4 * oi
                        en = en_ps.tile([128, KC], f32, tag="en", name="en")
                        for p in range(4):
                            c0 = qb_i * 256 + oi * 128
                            lhs = qtil[p][:, c0 : c0 + 128]
                            nc.tensor.matmul(
                                en[:, 0:w],
                                lhsT=lhs,
                                rhs=kPT[p][:, k0 : k0 + w],
                                start=(p == 0),
                                stop=(p == 3),
                            )
                        if has_alibi:
                            # en += c_g * dist, on DVE (frees PE cycles)
                            u0 = SEQ + k0 - qc0
                            nc.vector.scalar_tensor_tensor(
                                en[:, 0:w],
                                distt[:, u0 : u0 + w],
                                cgcolt[:, oi : oi + 1],
                                en[:, 0:w],
                                op0=mybir.AluOpType.mult,
                                op1=mybir.AluOpType.add,
                            )
                        nc.scalar.activation(
                            ex_s[ob][:, k0 : k0 + w],
                            en[:, 0:w],
                            EXP,
                            scale=SCALE,
                            accum_out=acc[:, 2 * c + oi : 2 * c + oi + 1],
                        )

                # softmax denominators; fold 1/sum into the mix2 matrices
                m2b = {}
                for oi, ob in enumerate(("A", "B")):
                    sm = smx_pool.tile([128, 2], f32, tag=f"sm{ob}", name=f"sm{ob}")
                    nc.vector.reduce_sum(
                        sm[:, 0:1],
                        acc.rearrange("p (c o) -> p c o", o=2)[:, 0 : NKC + 1, oi],
                        axis=X,
                    )
                    rc = smx_pool.tile([128, 1], f32, tag=f"rc{ob}", name=f"rc{ob}")
                    nc.vector.reciprocal(rc[:], sm[:, 0:1])
                    m2b[ob] = smx_pool.tile(
                        [128, 256], bf16, tag=f"m2b{ob}", name=f"m2b{ob}"
                    )
                    nc.vector.tensor_scalar_mul(
                        m2b[ob][:], w2pt[f"w2p{ob}"][:], rc[:, 0:1]
                    )

                # mix2 transposed: A'^T[k, (g, qi)], two k-subchunks per tile
                for jj in range(NKJ // 2):
                    m2 = m2_ps.tile([128, 512], f32, tag="m2", name="m2")
                    for j2 in range(2):
                        j = 2 * jj + j2
                        for ob in ("A", "B"):
                            nc.tensor.matmul(
                                m2[:, 256 * j2 : 256 * (j2 + 1)],
                                lhsT=ex_s[ob][:, 128 * j : 128 * (j + 1)],
                                rhs=m2b[ob][:],
                                start=(ob == "A"),
                                stop=(ob == "B"),
                            )
                    m2v = m2.rearrange("p (j g q) -> p j g q", j=2, g=HEADS, q=QB)
                    eng = nc.vector if jj % 2 == 0 else nc.scalar
                    ecopy(
                        eng,
                        aTv[:, 2 * jj : 2 * (jj + 1), :, qb_i * QB : (qb_i + 1) * QB],
                        m2v[:],
                    )
                # persistent k rows
                m2 = m2_ps.tile([128, 512], f32, tag="m2", name="m2p")
                for ob in ("A", "B"):
                    nc.tensor.matmul(
                        m2[0:NPERS, 0:256],
                        lhsT=ex_s[ob][:, SEQ:KT],
                        rhs=m2b[ob][:],
                        start=(ob == "A"),
                        stop=(ob == "B"),
                    )
                m2pv = m2[0:NPERS, 0:256].rearrange(
                    "p (g q) -> p g q", g=HEADS, q=QB
                )
                nc.vector.tensor_copy(
                    aTpv[:, :, qb_i * QB : (qb_i + 1) * QB], m2pv[:]
                )

            # ---- A@V for the group: attn^T[d, q] per out-head g ----
            at_s = []
            for g in range(HEADS):
                av = aux_ps.tile([128, QG], f32, tag="aux", name="av")
                for j in range(NKJ):
                    nc.tensor.matmul(
                        av[0:HD, :],
                        lhsT=vS[j][:, HD * g : HD * (g + 1)],
                        rhs=aTv[:, j, g, :],
                        start=(j == 0),
                        stop=False,
                    )
                nc.tensor.matmul(
                    av[0:HD, :],
                    lhsT=vSp[:, HD * g : HD * (g + 1)],
                    rhs=aTpv[:, g, :],
                    start=False,
                    stop=True,
                )
                ats = at_pool.tile([HD, QG], f16, tag=f"ats{g}", name=f"ats{g}")
                eng = nc.scalar if g % 2 == 0 else nc.vector
                ecopy(eng, ats[:], av[0:HD, :])
                at_s.append(ats)

            # ---- fc_out: 8 accumulated K=64 matmuls + rank-1 bias ----
            for sub in (0, 1):
                fp = m2_ps.tile([128, EMBED], f32, tag="m2", name="fp")
                for g in range(HEADS):
                    nc.tensor.matmul(
                        fp[:],
                        lhsT=at_s[g][:, 128 * sub : 128 * (sub + 1)],
                        rhs=fcw[g][:],
                        start=(g == 0),
                        stop=False,
                    )
                nc.tensor.matmul(
                    fp[:], lhsT=ones1[:], rhs=fcbt[:], start=False, stop=True
                )
                o_s = os_pool.tile([128, EMBED], f32, tag="os", name="o_s")
                nc.vector.tensor_copy(o_s[:], fp[:])
                q_row = qg * QG + sub * 128
                nc.sync.dma_start(out_d[q_row : q_row + 128, :], o_s[:])

    nc.compile()
    return nc


def _get_nc():
    if "nc" not in _CACHED:
        _CACHED["nc"] = build_bass()
    return _CACHED["nc"]


def kernel(
    values,
    keys,
    queries,
    mask,
    Wv,
    Wk,
    Wq,
    W_pre,
    W_post,
    p_keys,
    p_values,
    fc_w,
    fc_b,
):
    """Full-input entry point. mask is all-True per the problem spec
    (fill: ones) and is therefore not consumed on-device."""
    from concourse.bass_utils import run_bass_kernel_spmd

    qT = np.asarray(queries, np.float32).astype(np.float16).transpose(0, 2, 1)
    kT = np.asarray(keys, np.float32).astype(np.float16).transpose(0, 2, 1)
    vT = np.asarray(values, np.float32).astype(np.float16).transpose(0, 2, 1)
    consts = _host_consts(
        np.asarray(Wv, np.float32),
        np.asarray(Wk, np.float32),
        np.asarray(Wq, np.float32),
        np.asarray(W_pre, np.float32),
        np.asarray(W_post, np.float32),
        np.asarray(p_keys, np.float32),
        np.asarray(p_values, np.float32),
        np.asarray(fc_w, np.float32),
        np.asarray(fc_b, np.float32),
    )

    nc = _get_nc()
    in_maps = []
    for core in range(N_CORES):
        n, half = core // 2, core % 2
        qbase = half * SQ
        m = {
            "qT16": np.ascontiguousarray(qT[n, :, qbase : qbase + SQ]),
            "kT16": np.ascontiguousarray(kT[n]),
            "vT16": np.ascontiguousarray(vT[n]),
            "dist": _dist_table(qbase),
        }
        m.update(consts)
        in_maps.append(m)

    global _last_in_maps
    _last_in_maps = in_maps
    res = run_bass_kernel_spmd(nc, in_maps, core_ids=list(range(N_CORES)))
    out = np.empty((N_BATCH, SEQ, EMBED), np.float32)
    for core in range(N_CORES):
        n, half = core // 2, core % 2
        out[n, half * SQ : (half + 1) * SQ, :] = res.results[core]["out"]
    return out



# revision 14
# speedup vs baseline: 1.1316x; 1.1316x over previous
"""TRN2 Bass/Tile kernel for NeoTCNAttention (talking-heads attention with
ALiBi + persistent memory), SPMD over 8 NeuronCores.

Sharding: data-parallel over batch N=4 x 2 halves of the query axis
(each core: one batch element, 1024 query positions, full keys/values).
No collectives: every core computes a disjoint slab of the output.

v3 design (per core), all activations fp16:
  - 16-query blocks: rows (8 out-heads x 16 q) so one 128-row tile holds
    all heads; the post-softmax talking-heads transpose is a single
    K=128 matmul per 128-key chunk (no bank accumulation).
  - scores: W_pre folded into q~ (8 scaled copies per pair), 4 pair
    matmuls per 512-key chunk into 2-bank PSUM tiles; one big exp
    ACTIVATE per [128,1024] tile.
  - ALiBi: host-precomputed exp(SCALE*c_g*dist) FACTOR table, row-
    normalized by alpha_r = 1/max_fac_r to fit fp16 (alpha enters the
    persistent slots via the exp bias, and cancels in the softmax
    normalization). Applied in-place by one fused DVE
    scalar_tensor_tensor that also emits the softmax row sums.
  - persistent-slot energies batched 8 blocks per PSUM bank -> one exp.
  - mix2 (post-softmax talking heads + transpose + A2/S) as K=128
    matmuls; A@V per 128-query group; fc_out with rank-1 bias.
"""

import math

import numpy as np

# ---- problem constants (hardcoded per spec) ----
N_BATCH = 4
SEQ = 2048
EMBED = 512
HEADS = 8
HD = 64
NPERS = 16
KT = SEQ + NPERS  # 2064
ALIBI_ALPHA = 1.25
START_I = 1
N_CORES = 8
SQ = 1024  # query positions per core
SCALE = 1.0 / math.sqrt(EMBED)

QB = 16            # query positions per block: rows = (8 g, 16 qi)
NQB = SQ // QB     # 64 blocks
QG = 128           # query positions per A@V group
NQG = SQ // QG     # 8 groups
BPG = QG // QB     # 8 blocks per group
NKJ = SEQ // 128   # 16 key subchunks for mix2/A@V
DIST_W = SEQ + KT  # 4112 factor-table width

_CACHED = {}
_last_in_maps = None


def _host_consts(Wv, Wk, Wq, W_pre, W_post, p_keys, p_values, fc_w, fc_b):
    """Derived constant tensors shipped to the device (layout prep only)."""
    f16 = np.float16
    Z = np.zeros((HD, HD), np.float32)
    c = {}
    c["wq2"] = np.block([[Wq.T, Z], [Z, Wq.T]]).astype(f16)  # [128, 128]
    c["wk2"] = np.block([[Wk.T, Z], [Z, Wk.T]]).astype(f16)
    c["wv2"] = np.block([[Wv.T, Z], [Z, Wv.T]]).astype(f16)
    c["pk16"] = np.ascontiguousarray(p_keys[:, 0, :].T).astype(f16)  # [64, 16]
    c["pv16"] = np.tile(p_values[:, 0, :], (1, HEADS)).astype(f16)   # [16, 512]

    # W_pre scaling columns for the q~ build: col 4*g + p scales pair p's
    # rows (head 2p in partitions 0-63, 2p+1 in 64-127) for out-head g.
    wpcol = np.zeros((128, HEADS * 4), np.float32)
    for g in range(HEADS):
        for p in range(4):
            wpcol[0:HD, 4 * g + p] = W_pre[g, 2 * p]
            wpcol[HD:128, 4 * g + p] = W_pre[g, 2 * p + 1]
    c["wpcol"] = wpcol

    # attention amp A2 (keeps m2b = A2*W_post/S out of fp16 subnormals)
    rowmax = float(np.abs(W_post).sum(axis=1).max())
    a2 = 2.0 ** math.floor(math.log2(2000.0 / rowmax))

    # w2p (x A2): [16g+qi, 16g2+qi] = A2*W_post[g2, g]
    idx = np.arange(QB)
    w2p = np.zeros((128, 128), np.float32)
    for g in range(HEADS):
        for g2 in range(HEADS):
            w2p[QB * g + idx, QB * g2 + idx] = a2 * W_post[g2, g]
    c["w2pt"] = w2p.astype(f16)
    c["atsc"] = np.full((HD, 1), 1.0 / a2, np.float32)

    c["fcwT"] = np.ascontiguousarray(fc_w.T).astype(f16)  # [512, 512]
    c["fcb"] = fc_b.reshape(1, EMBED).astype(f16)
    return c


def _factor_table(W_pre, qbase: int):
    """F[16*g + qi, u] = alpha_r * exp(SCALE*c_g*|qbase + qi - (u - SEQ)|)
    with alpha_r = 1/max(1, max_u fac) so the fp16 table never overflows;
    also returns ln(alpha) [128, 1] (the persistent-slot exp bias)."""
    slopes = 2.0 ** (
        -ALIBI_ALPHA * (np.arange(1, HEADS + 1, dtype=np.float64) + START_I)
    )
    cg = -(np.asarray(W_pre, np.float64) @ slopes)  # [8]
    u = np.arange(DIST_W)
    qi = np.arange(QB)
    dist = np.abs(qbase + qi[:, None] - (u[None, :] - SEQ))  # [16, 4112]
    t = np.exp(SCALE * cg[:, None, None] * dist[None, :, :]).reshape(128, DIST_W)
    # normalize by the max factor over REALIZED distances only (the table
    # corners are never read); unread entries are clipped to fp16 range.
    d_real = np.maximum(qbase + SQ - QB + qi, SEQ - 1 - qbase - qi)  # [16]
    fmax = np.exp(SCALE * np.maximum(cg[:, None], 0.0) * d_real[None, :])
    alpha = (1.0 / np.maximum(fmax, 1.0)).reshape(128, 1)
    t = np.minimum(t * alpha, 6.0e4)
    return t.astype(np.float16), np.log(alpha).astype(np.float32)


def build_bass():
    import concourse.mybir as mybir
    import concourse.tile as tile
    from concourse import bacc
    from contextlib import ExitStack

    f32 = mybir.dt.float32
    f16 = mybir.dt.float16
    EXP = mybir.ActivationFunctionType.Exp
    COPY = mybir.ActivationFunctionType.Copy
    X = mybir.AxisListType.X
    MUL = mybir.AluOpType.mult
    ADD = mybir.AluOpType.add

    nc = bacc.Bacc(
        "TRN2", target_bir_lowering=False, debug=False, num_devices=N_CORES
    )

    # ---- DRAM I/O ----
    qT_d = nc.dram_tensor("qT16", [EMBED, SQ], f16, kind="ExternalInput").ap()
    kT_d = nc.dram_tensor("kT16", [EMBED, SEQ], f16, kind="ExternalInput").ap()
    vT_d = nc.dram_tensor("vT16", [EMBED, SEQ], f16, kind="ExternalInput").ap()
    wq2_d = nc.dram_tensor("wq2", [128, 128], f16, kind="ExternalInput").ap()
    wk2_d = nc.dram_tensor("wk2", [128, 128], f16, kind="ExternalInput").ap()
    wv2_d = nc.dram_tensor("wv2", [128, 128], f16, kind="ExternalInput").ap()
    pk16_d = nc.dram_tensor("pk16", [HD, NPERS], f16, kind="ExternalInput").ap()
    pv16_d = nc.dram_tensor("pv16", [NPERS, EMBED], f16, kind="ExternalInput").ap()
    wpcol_d = nc.dram_tensor(
        "wpcol", [128, HEADS * 4], f32, kind="ExternalInput"
    ).ap()
    w2p_d = nc.dram_tensor("w2pt", [128, 128], f16, kind="ExternalInput").ap()
    ftab_d = nc.dram_tensor("ftab", [128, DIST_W], f16, kind="ExternalInput").ap()
    lna_d = nc.dram_tensor("lna", [128, 1], f32, kind="ExternalInput").ap()
    atsc_d = nc.dram_tensor("atsc", [HD, 1], f32, kind="ExternalInput").ap()
    fcwT_d = nc.dram_tensor("fcwT", [EMBED, EMBED], f16, kind="ExternalInput").ap()
    fcb_d = nc.dram_tensor("fcb", [1, EMBED], f16, kind="ExternalInput").ap()
    out_d = nc.dram_tensor("out", [SQ, EMBED], f32, kind="ExternalOutput").ap()

    def ecopy(eng, out, in_):
        if eng is nc.scalar:
            eng.copy(out, in_)
        else:
            eng.tensor_copy(out, in_)

    with tile.TileContext(nc) as tc, ExitStack() as ctx:
        const_pool = ctx.enter_context(tc.tile_pool(name="const", bufs=1))
        xT_pool = ctx.enter_context(tc.tile_pool(name="xT", bufs=1))
        qt_pool = ctx.enter_context(tc.tile_pool(name="qt", bufs=2))
        xraw_pool = ctx.enter_context(tc.tile_pool(name="xraw", bufs=2))
        aexp_pool = ctx.enter_context(tc.tile_pool(name="aexp", bufs=2))
        smx_pool = ctx.enter_context(tc.tile_pool(name="smx", bufs=2))
        aT_pool = ctx.enter_context(tc.tile_pool(name="aTp", bufs=2))
        at_pool = ctx.enter_context(tc.tile_pool(name="atp", bufs=2))
        os_pool = ctx.enter_context(tc.tile_pool(name="osp", bufs=2))

        # PSUM: en 2x[128,1024] (4 banks) + m2 [128,512]x2 (2) + av/enP
        # tag-shared x2 (2) = 8 banks exactly.
        en_ps = ctx.enter_context(tc.tile_pool(name="en_ps", bufs=1, space="PSUM"))
        m2_ps = ctx.enter_context(tc.tile_pool(name="m2_ps", bufs=2, space="PSUM"))
        av_ps = ctx.enter_context(tc.tile_pool(name="av_ps", bufs=2, space="PSUM"))

        # ---- constants ----
        def cload(ap_d, shape, dtype, nm):
            t = const_pool.tile(shape, dtype, tag=nm, name=nm)
            nc.sync.dma_start(t[:], ap_d)
            return t

        wq2t = cload(wq2_d, [128, 128], f16, "wq2t")
        wk2t = cload(wk2_d, [128, 128], f16, "wk2t")
        wv2t = cload(wv2_d, [128, 128], f16, "wv2t")
        pk16t = cload(pk16_d, [HD, NPERS], f16, "pk16t")
        vSp = cload(pv16_d, [NPERS, EMBED], f16, "vSp")
        wpcolt = cload(wpcol_d, [128, HEADS * 4], f32, "wpcolt")
        w2pt = cload(w2p_d, [128, 128], f16, "w2pt")
        ftab = cload(ftab_d, [128, DIST_W], f16, "ftab")
        lnat = cload(lna_d, [128, 1], f32, "lnat")
        atsct = cload(atsc_d, [HD, 1], f32, "atsct")
        fcw = []
        for cc in range(HEADS):
            t = const_pool.tile([HD, EMBED], f16, tag=f"fcw{cc}", name=f"fcw{cc}")
            nc.sync.dma_start(t[:], fcwT_d[HD * cc : HD * (cc + 1), :])
            fcw.append(t)
        fcbt = cload(fcb_d, [1, EMBED], f16, "fcbt")
        ones1 = const_pool.tile([1, 128], f16, tag="ones1", name="ones1")
        nc.vector.memset(ones1[:], 1.0)

        # ---- persistent activation tiles ----
        qPT = [
            xT_pool.tile([128, SQ], f16, tag=f"qPT{p}", name=f"qPT{p}")
            for p in range(4)
        ]
        kPT = [
            xT_pool.tile([128, KT], f16, tag=f"kPT{p}", name=f"kPT{p}")
            for p in range(4)
        ]
        # vS: [128 k-in-chunk, (j 16, (g,d) 512)]
        vS = xT_pool.tile([128, NKJ * EMBED], f16, tag="vS", name="vS")
        vSv = vS.rearrange("p (j e) -> p j e", j=NKJ, e=EMBED)

        for p in range(4):
            nc.vector.tensor_copy(kPT[p][0:HD, SEQ:KT], pk16t[:])
            nc.vector.tensor_copy(kPT[p][HD:128, SEQ:KT], pk16t[:])

        # ---- projections (block-diag weights, K=128) ----
        for src_d, nrows, w2, dstT in ((qT_d, SQ, wq2t, qPT), (kT_d, SEQ, wk2t, kPT)):
            for p in range(4):
                rT = xraw_pool.tile([128, SEQ], f16, tag="xr", name="rT")
                nc.sync.dma_start(rT[:, 0:nrows], src_d[128 * p : 128 * (p + 1), :])
                for c in range(nrows // 512):
                    ps = m2_ps.tile([128, 512], f32, tag="m2", name="psp")
                    nc.tensor.matmul(
                        ps[:],
                        lhsT=w2[:],
                        rhs=rT[:, 512 * c : 512 * (c + 1)],
                        start=True,
                        stop=True,
                    )
                    nc.scalar.copy(dstT[p][:, 512 * c : 512 * (c + 1)], ps[:])
        # v: project + transpose back to [k, (g,d)] via stationary x-chunks
        for p in range(4):
            rT = xraw_pool.tile([128, SEQ], f16, tag="xr", name="rTv")
            nc.sync.dma_start(rT[:], vT_d[128 * p : 128 * (p + 1), :])
            for jg in range(4):
                ps = m2_ps.tile([128, 512], f32, tag="m2", name="psv")
                for j2 in range(4):
                    j = 4 * jg + j2
                    nc.tensor.matmul(
                        ps[:, 128 * j2 : 128 * (j2 + 1)],
                        lhsT=rT[:, 128 * j : 128 * (j + 1)],
                        rhs=wv2t[:],
                        start=True,
                        stop=True,
                    )
                dv = vSv[:, 4 * jg : 4 * jg + 4, 128 * p : 128 * (p + 1)]
                ecopy(nc.scalar if (p + jg) % 2 else nc.vector, dv, ps.rearrange(
                    "p (j n) -> p j n", j=4, n=128))

        # ---- main attention loop (1-block software pipelining) ----
        state = {}

        def emit_qtil_ops(pg, lo, hi):
            """q~ build ops [lo, hi) for group-pair pg (2 groups, 256 q):
            per pair, cols (b 16, g 8, q 16); op index = 8*p + g."""
            if ("qt", pg) not in state:
                state[("qt", pg)] = [
                    qt_pool.tile([128, 2048], f16, tag=f"qt{p}", name=f"qt{p}")
                    for p in range(4)
                ]
            qt = state[("qt", pg)]
            for op in range(lo, hi):
                p, g = op // HEADS, op % HEADS
                qv = qt[p].rearrange(
                    "p (b g q) -> p b g q", b=2 * BPG, g=HEADS, q=QB
                )
                src = qPT[p][:, 256 * pg : 256 * pg + 256].rearrange(
                    "p (b q) -> p b q", b=2 * BPG, q=QB
                )
                nc.vector.tensor_scalar_mul(
                    qv[:, :, g, :], src[:], wpcolt[:, 4 * g + p : 4 * g + p + 1]
                )

        def emit_scores(b):
            qt = state[("qt", b // (2 * BPG))]
            bi2 = b % (2 * BPG)
            enA = en_ps.tile([128, 1024], f32, tag="enA", name="enA")
            enB = en_ps.tile([128, 1024], f32, tag="enB", name="enB")
            enP = m2_ps.tile([128, NPERS], f32, tag="m2", name="enP")
            for p in range(4):
                lhsT = qt[p][:, 128 * bi2 : 128 * (bi2 + 1)]
                st, sp_ = (p == 0), (p == 3)
                for c, dst in ((0, enA[:, 0:512]), (1, enA[:, 512:1024]),
                               (2, enB[:, 0:512]), (3, enB[:, 512:1024])):
                    nc.tensor.matmul(
                        dst,
                        lhsT=lhsT,
                        rhs=kPT[p][:, 512 * c : 512 * (c + 1)],
                        start=st,
                        stop=sp_,
                    )
                nc.tensor.matmul(
                    enP[:],
                    lhsT=lhsT,
                    rhs=kPT[p][:, SEQ:KT],
                    start=st,
                    stop=sp_,
                )
            state[("en", b)] = (enA, enB, enP)

        def emit_softmax(b):
            enA, enB, enP = state.pop(("en", b))
            aexp = aexp_pool.tile([128, KT], f16, tag="aexp", name="aexp")
            nc.scalar.activation(aexp[:, 0:1024], enA[:], EXP, scale=SCALE)
            nc.scalar.activation(aexp[:, 1024:2048], enB[:], EXP, scale=SCALE)
            nc.scalar.activation(
                aexp[:, SEQ:KT], enP[:], EXP, bias=lnat[:, 0:1], scale=SCALE
            )
            S1 = smx_pool.tile([128, 1], f32, tag="S1", name="S1")
            u0 = SEQ - QB * b
            nc.vector.scalar_tensor_tensor(
                aexp[:, 0:SEQ],
                aexp[:, 0:SEQ],
                1.0,
                ftab[:, u0 : u0 + SEQ],
                op0=MUL,
                op1=MUL,
                accum_out=S1[:, 0:1],
            )
            sp = smx_pool.tile([128, 1], f32, tag="sp", name="sp")
            nc.vector.reduce_sum(sp[:], aexp[:, SEQ:KT], axis=X)
            Sr = smx_pool.tile([128, 1], f32, tag="Sr", name="Sr")
            nc.vector.scalar_tensor_tensor(
                Sr[:], S1[:], 1.0, sp[:], op0=MUL, op1=ADD
            )
            rc = smx_pool.tile([128, 1], f32, tag="rc", name="rc")
            nc.vector.reciprocal(rc[:], Sr[:, 0:1])
            m2b = smx_pool.tile([128, 128], f16, tag="m2b", name="m2b")
            nc.vector.tensor_scalar_mul(m2b[:], w2pt[:], rc[:, 0:1])
            state[(b, "n")] = (aexp, m2b)

        def emit_mix2(b):
            aexp, m2b = state.pop((b, "n"))
            aTv, aTpv = state[("aT", b // BPG)]
            bi = b % BPG
            for grp in range(4):
                m2 = m2_ps.tile([128, 512], f32, tag="m2", name="m2")
                for t2 in range(4):
                    j = 4 * grp + t2
                    nc.tensor.matmul(
                        m2[:, 128 * t2 : 128 * (t2 + 1)],
                        lhsT=aexp[:, 128 * j : 128 * (j + 1)],
                        rhs=m2b[:],
                        start=True,
                        stop=True,
                    )
                dv = aTv[:, 4 * grp : 4 * grp + 4, :, QB * bi : QB * (bi + 1)]
                ecopy(nc.scalar if grp % 2 else nc.vector, dv, m2.rearrange(
                    "p (j g q) -> p j g q", j=4, g=HEADS, q=QB))
            m2 = m2_ps.tile([128, 512], f32, tag="m2", name="m2p")
            nc.tensor.matmul(
                m2[0:NPERS, 0:128],
                lhsT=aexp[:, SEQ:KT],
                rhs=m2b[:],
                start=True,
                stop=True,
            )
            nc.vector.tensor_copy(
                aTpv[:, :, QB * bi : QB * (bi + 1)],
                m2[0:NPERS, 0:128].rearrange("p (g q) -> p g q", g=HEADS, q=QB),
            )

        def emit_av(qg, g):
            aTv, aTpv = state[("aT", qg)]
            av = av_ps.tile([HD, QG], f32, tag="avp", name="av")
            for j in range(NKJ):
                nc.tensor.matmul(
                    av[:],
                    lhsT=vSv[:, j, HD * g : HD * (g + 1)],
                    rhs=aTv[:, j, g, :],
                    start=(j == 0),
                    stop=False,
                )
            nc.tensor.matmul(
                av[:],
                lhsT=vSp[:, HD * g : HD * (g + 1)],
                rhs=aTpv[:, g, :],
                start=False,
                stop=True,
            )
            ats = at_pool.tile([HD, QG], f16, tag=f"ats{g}", name=f"ats{g}")
            nc.scalar.activation(ats[:], av[:], COPY, scale=atsct[:, 0:1])
            state[("ats", qg, g)] = ats

        def emit_fc(qg):
            fp = m2_ps.tile([128, 512], f32, tag="m2", name="fp")
            for g in range(HEADS):
                ats = state.pop(("ats", qg, g))
                nc.tensor.matmul(
                    fp[:],
                    lhsT=ats[:],
                    rhs=fcw[g][:],
                    start=(g == 0),
                    stop=False,
                )
            nc.tensor.matmul(
                fp[:], lhsT=ones1[:], rhs=fcbt[:], start=False, stop=True
            )
            o_s = os_pool.tile([128, EMBED], f32, tag="os", name="o_s")
            nc.vector.tensor_copy(o_s[:], fp[:])
            nc.sync.dma_start(out_d[qg * QG : qg * QG + 128, :], o_s[:])
            del state[("aT", qg)]

        emit_qtil_ops(0, 0, 32)
        for b in range(NQB):
            if b % BPG == 0:
                aT = aT_pool.tile(
                    [128, NKJ * HEADS * QG], f16, tag="aT", name="aT"
                )
                aTv = aT.rearrange("p (j g q) -> p j g q", j=NKJ, g=HEADS, q=QG)
                aTp = aT_pool.tile([NPERS, HEADS * QG], f16, tag="aTq", name="aTq")
                aTpv = aTp.rearrange("p (g q) -> p g q", g=HEADS, q=QG)
                state[("aT", b // BPG)] = (aTv, aTpv)
            emit_scores(b)
            emit_softmax(b)
            # q~ for group-pair pg built spread over the prior 16 blocks
            pg = b // (2 * BPG) + 1
            if pg < NQG // 2:
                emit_qtil_ops(pg, 2 * (b % (2 * BPG)), 2 * (b % (2 * BPG)) + 2)
            if b >= 1:
                emit_mix2(b - 1)
            if b >= BPG:
                qg, r = b // BPG - 1, b % BPG
                if 1 <= r <= 4:
                    emit_av(qg, 2 * (r - 1))
                    emit_av(qg, 2 * r - 1)
                elif r == 5:
                    emit_fc(qg)
        emit_mix2(NQB - 1)
        for g in range(HEADS):
            emit_av(NQG - 1, g)
        emit_fc(NQG - 1)

    nc.compile()
    return nc


def _get_nc():
    if "nc" not in _CACHED:
        _CACHED["nc"] = build_bass()
    return _CACHED["nc"]


def kernel(
    values,
    keys,
    queries,
    mask,
    Wv,
    Wk,
    Wq,
    W_pre,
    W_post,
    p_keys,
    p_values,
    fc_w,
    fc_b,
):
    """Full-input entry point. mask is all-True per the problem spec
    (fill: ones) and is therefore not consumed on-device."""
    from concourse.bass_utils import run_bass_kernel_spmd

    qT = np.asarray(queries, np.float32).astype(np.float16).transpose(0, 2, 1)
    kT = np.asarray(keys, np.float32).astype(np.float16).transpose(0, 2, 1)
    vT = np.asarray(values, np.float32).astype(np.float16).transpose(0, 2, 1)
    W_pre = np.asarray(W_pre, np.float32)
    consts = _host_consts(
        np.asarray(Wv, np.float32),
        np.asarray(Wk, np.float32),
        np.asarray(Wq, np.float32),
        W_pre,
        np.asarray(W_post, np.float32),
        np.asarray(p_keys, np.float32),
        np.asarray(p_values, np.float32),
        np.asarray(fc_w, np.float32),
        np.asarray(fc_b, np.float32),
    )

    nc = _get_nc()
    ftabs = {qb: _factor_table(W_pre, qb) for qb in (0, SQ)}
    in_maps = []
    for core in range(N_CORES):
        n, half = core // 2, core % 2
        qbase = half * SQ
        ft, lna = ftabs[qbase]
        m = {
            "qT16": np.ascontiguousarray(qT[n, :, qbase : qbase + SQ]),
            "kT16": np.ascontiguousarray(kT[n]),
            "vT16": np.ascontiguousarray(vT[n]),
            "ftab": ft,
            "lna": lna,
        }
        m.update(consts)
        in_maps.append(m)

    global _last_in_maps
    _last_in_maps = in_maps
    res = run_bass_kernel_spmd(nc, in_maps, core_ids=list(range(N_CORES)))
    out = np.empty((N_BATCH, SEQ, EMBED), np.float32)
    for core in range(N_CORES):
        n, half = core // 2, core % 2
        out[n, half * SQ : (half + 1) * SQ, :] = res.results[core]["out"]
    return out
